# revision 1
# baseline (speedup 1.0000x reference)
"""Trainium2 Bass kernel for nn_Aggregator (BN1d + Swish + Linear + relevance-weighted head sum).

out[b, :] = sum_h w[b,h] * (silu(x[b,h,:] * inv + shift) @ W.T + bias)
          = (sum_h w[b,h] * silu(x[b,h,:] * inv + shift)) @ W.T + (sum_h w[b,h]) * bias

Data parallel over 8 NeuronCores: batch dim B split 8 ways, all params replicated.

Per-core layout (B_loc=1024 b-values -> 8192 flat rows of [512]):
  - 8 "blocks" of 128 b-values; each block = 8 row-tiles of [128 rows, 512].
  - BN affine: DVE mul + DVE/GPSIMD add (params broadcast across partitions).
  - Swish: ScalarE Silu.
  - Weighted head-sum: TensorE matmul with a per-tile staircase matrix
    Wagg[p, b'] = w[row p] * [b' == 16*j + p//8], accumulated over the 8
    row-tiles of a block in PSUM -> g[128 b, 512].
  - g transposed on TensorE (via identity), then g @ W.T on TensorE.
  - bias: out = psum + sumw[b] * bias via one scalar_tensor_tensor.
"""

import os
from contextlib import ExitStack

import numpy as np

import concourse.bacc as bacc
import concourse.mybir as mybir
import concourse.tile as tile
from concourse.bass_utils import run_bass_kernel_spmd
from concourse.mybir import AluOpType

N_CORES = 8
B, H, D, DO = 8192, 8, 512, 256
B_LOC = B // N_CORES            # 1024 b-values per core
ROWS = B_LOC * H                # 8192 flat rows per core
NBLK = B_LOC // 128             # 8 blocks of 128 b-values
EPS = 1e-5
FP = mybir.dt.float32

# Which row-tiles' BN-mul runs on GPSIMD (rest on DVE). Tuned from traces.
GP_MUL_JS = (1, 2, 4, 6)
FPR = mybir.dt.float32r
BF = mybir.dt.bfloat16


ALL_STAGES = frozenset({"bnmul", "bnadd", "silu", "agg", "tail"})


def build_kernel(
    nblk: int = NBLK,
    stages: frozenset = ALL_STAGES,
    bf16_bn: bool = True,
    repeat: int = 1,
):
    """repeat>1 re-runs the whole block loop (same I/O) for slope timing."""
    TB = BF if bf16_bn else FP
    nc = bacc.Bacc(
        "TRN2",
        target_bir_lowering=False,
        debug=False,
        num_devices=N_CORES,
    )

    x_d = nc.dram_tensor("x", (nblk, H, 128, D), FP, kind="ExternalInput")
    # w laid out host-side as [128, ntiles]: column k holds w for flat rows 128k..128k+127
    w_d = nc.dram_tensor("w", (128, nblk * H), FP, kind="ExternalInput")
    sumw_d = nc.dram_tensor("sumw", (128, nblk), FP, kind="ExternalInput")
    invb_d = nc.dram_tensor("invb", (128, D), FP, kind="ExternalInput")
    shiftb_d = nc.dram_tensor("shiftb", (128, D), TB, kind="ExternalInput")
    biasb_d = nc.dram_tensor("biasb", (128, DO), FP, kind="ExternalInput")
    # W.T chunked: wt[p, c*DO + m] = W[m, 128c + p]
    wt_d = nc.dram_tensor("wt", (128, 4 * DO), FPR, kind="ExternalInput")
    # one shared staircase window: astair[p, q] = (q == 112 + p//8);
    # variant j is the slice [:, 112-16j : 240-16j]
    astair_d = nc.dram_tensor("astair", (128, 240), FP, kind="ExternalInput")
    ident_d = nc.dram_tensor("ident", (128, 128), FP, kind="ExternalInput")
    out_d = nc.dram_tensor("out", (nblk, 128, DO), FP, kind="ExternalOutput")

    with tile.TileContext(nc) as tc, ExitStack() as ctx:
        const = ctx.enter_context(tc.tile_pool(name="const", bufs=1))
        xpool = ctx.enter_context(tc.tile_pool(name="xin", bufs=5))
        # DMA issue order tuned for fill: first x slices, then the small
        # tiles the first row-tiles depend on, then the rest of block 0,
        # then the bulkier tail constants.
        xt0 = xpool.tile([128, H * D], FP, tag="xt")
        for j in range(2):
            nc.sync.dma_start(xt0[:, j * D : (j + 1) * D], x_d.ap()[0][j])
        invb = const.tile([128, D], FP)
        nc.sync.dma_start(invb[:], invb_d.ap())
        shiftb = const.tile([128, D], TB)
        nc.sync.dma_start(shiftb[:], shiftb_d.ap())
        astair = const.tile([128, 240], FP)
        nc.sync.dma_start(astair[:], astair_d.ap())
        w_sb = const.tile([128, nblk * H], FP)
        nc.sync.dma_start(w_sb[:], w_d.ap())
        for j in range(2, H):
            nc.sync.dma_start(xt0[:, j * D : (j + 1) * D], x_d.ap()[0][j])

        biasb = const.tile([128, DO], FP)
        nc.sync.dma_start(biasb[:], biasb_d.ap())
        wt = const.tile([128, 4 * DO], FPR)
        nc.sync.dma_start(wt[:], wt_d.ap())
        ident = const.tile([128, 128], FP)
        nc.sync.dma_start(ident[:], ident_d.ap())
        sumw = const.tile([128, nblk], FP)
        nc.sync.dma_start(sumw[:], sumw_d.ap())
        tpool = ctx.enter_context(tc.tile_pool(name="tmp", bufs=10))
        spool = ctx.enter_context(tc.tile_pool(name="act", bufs=6))
        wgpool = ctx.enter_context(tc.tile_pool(name="wagg", bufs=6))
        gpool = ctx.enter_context(tc.tile_pool(name="g", bufs=2))
        gtpool = ctx.enter_context(tc.tile_pool(name="gt", bufs=2))
        opool = ctx.enter_context(tc.tile_pool(name="o", bufs=2))
        psg = ctx.enter_context(tc.tile_pool(name="psg", bufs=3, space="PSUM"))
        pst = ctx.enter_context(tc.tile_pool(name="pst", bufs=2, space="PSUM"))
        pso = ctx.enter_context(tc.tile_pool(name="pso", bufs=2, space="PSUM"))

        for rep in range(repeat):
          for n in range(nblk):
            if n == 0 and rep == 0:
                xt = xt0
            else:
                # split loads: halves mid-stream, quarters for the last block
                # so its compute chain starts as early as possible
                xt = xpool.tile([128, H * D], FP, tag="xt")
                nsplit = 4 if n == nblk - 1 else 2
                js_per = H // nsplit
                for sp in range(nsplit):
                    j0 = sp * js_per
                    nc.sync.dma_start(
                        xt[:, j0 * D : (j0 + js_per) * D].rearrange(
                            "p (j d) -> p j d", j=js_per
                        ),
                        x_d.ap()[n][j0 : j0 + js_per].rearrange("j p d -> p j d"),
                    )
            pg = psg.tile([128, D], FP)
            for k in range(H // 2):
                # pair of row-tiles j=2k, 2k+1 processed at [128, 2*D]
                t1 = tpool.tile([128, 2 * D], TB, tag="t1")
                for v in range(2):
                    j = 2 * k + v
                    if "bnmul" in stages:
                        # late blocks lean DVE so the GP doesn't pace the tail
                        if n == nblk - 1:
                            on_gp = j in (0, 2)
                        elif n == nblk - 2:
                            on_gp = j in (0, 2, 4, 6)
                        else:
                            on_gp = j in GP_MUL_JS
                        eng = nc.gpsimd if on_gp else nc.vector
                        eng.tensor_tensor(
                            t1[:, v * D : (v + 1) * D],
                            xt[:, j * D : (j + 1) * D],
                            invb[:],
                            AluOpType.mult,
                        )
                cur = t1[:]
                if "bnadd" in stages:
                    t2 = tpool.tile([128, 2 * D], TB, tag="t2")
                    nc.vector.tensor_tensor(
                        t2[:].rearrange("p (r d) -> p r d", r=2),
                        cur.rearrange("p (r d) -> p r d", r=2),
                        shiftb[:].unsqueeze(1).broadcast_to([128, 2, D]),
                        AluOpType.add,
                    )
                    cur = t2[:]
                if "silu" in stages:
                    s = spool.tile([128, 2 * D], FPR)
                    nc.scalar.activation(
                        s[:], cur, mybir.ActivationFunctionType.Silu
                    )
                    cur = s[:]
                if "agg" in stages:
                    # Wagg[p, c] = w[row p] * (c == 16*j + p//8); all 8
                    # row-tiles of the block accumulate into pg[:, :].
                    for v in range(2):
                        j = 2 * k + v
                        wg = wgpool.tile([128, 128], FPR)
                        wg_eng = nc.gpsimd if v == 0 else nc.vector
                        wg_eng.tensor_scalar_mul(
                            wg[:],
                            astair[:, 112 - 16 * j : 240 - 16 * j],
                            w_sb[:, n * H + j : n * H + j + 1],
                        )
                        nc.tensor.matmul(
                            pg[:],
                            wg[:],
                            cur[:, v * D : (v + 1) * D],
                            start=(j == 0),
                            stop=(j == H - 1),
                        )
            if "tail" not in stages:
                continue
            g = gpool.tile([128, D], FP)
            nc.scalar.copy(g[:], pg[:])
            pt = pst.tile([128, D], FP)
            for c in range(4):
                nc.tensor.transpose(
                    pt[:, c * 128 : (c + 1) * 128],
                    g[:, c * 128 : (c + 1) * 128],
                    ident[:],
                )
            gt = gtpool.tile([128, D], FPR)
            nc.scalar.copy(gt[:], pt[:])
            po = pso.tile([128, DO], FP)
            for c in range(4):
                nc.tensor.matmul(
                    po[:],
                    gt[:, c * 128 : (c + 1) * 128],
                    wt[:, c * DO : (c + 1) * DO],
                    start=(c == 0),
                    stop=(c == 3),
                )
            o = opool.tile([128, DO], FP)
            nc.vector.scalar_tensor_tensor(
                o[:], biasb[:], sumw[:, n : n + 1], po[:],
                AluOpType.mult, AluOpType.add,
            )
            # stores go out the ACT HWDGE queue so a store waiting on compute
            # never blocks the next x load in the SP queue's FIFO
            nc.scalar.dma_start(out_d.ap()[n], o[:])

    nc.compile()
    return nc


def make_host_inputs(x_np, w_np, gamma, beta, mean, var, W, b, nblk: int = NBLK, bf16_bn: bool = True):
    """Build the per-core input maps (host-side layout prep only)."""
    import ml_dtypes

    inv = (gamma / np.sqrt(var + EPS)).astype(np.float32)
    shift = (beta - mean * inv).astype(np.float32)
    invb = np.ascontiguousarray(np.broadcast_to(inv, (128, D)))
    sdt = ml_dtypes.bfloat16 if bf16_bn else np.float32
    shiftb = np.ascontiguousarray(np.broadcast_to(shift.astype(sdt), (128, D)))
    biasb = np.ascontiguousarray(np.broadcast_to(b.astype(np.float32), (128, DO)))
    wt = np.ascontiguousarray(
        W.astype(np.float32).T.reshape(4, 128, DO).transpose(1, 0, 2).reshape(128, 4 * DO)
    )
    p = np.arange(128)
    astair = np.zeros((128, 240), dtype=np.float32)
    astair[p, 112 + p // 8] = 1.0
    ident = np.eye(128, dtype=np.float32)

    rows_loc = nblk * H * 128
    b_loc = nblk * 128
    in_maps = []
    for core in range(N_CORES):
        b0 = core * B_LOC
        # flat row order: row = ((n*H + j)*128 + p) -> x tile [n, j, p, d]
        x_loc = np.ascontiguousarray(x_np[b0 : b0 + b_loc].reshape(nblk, H, 128, D))
        w_flat = w_np[b0 : b0 + b_loc].reshape(rows_loc).astype(np.float32)
        w_sb = np.ascontiguousarray(w_flat.reshape(nblk * H, 128).T)
        sumw = w_np[b0 : b0 + b_loc].sum(axis=1).astype(np.float32)
        sumw_sb = np.ascontiguousarray(sumw.reshape(nblk, 128).T)
        in_maps.append(
            {
                "x": x_loc,
                "w": w_sb,
                "sumw": sumw_sb,
                "invb": invb,
                "shiftb": shiftb,
                "biasb": biasb,
                "wt": wt,
                "astair": astair,
                "ident": ident,
            }
        )
    return in_maps


_NC_CACHE = None
LAST_RESULT = None


def make_runner(nc, in_maps):
    """Build a reusable jitted SPMD callable with device-resident inputs.

    Mirrors bass2jax.run_bass_via_pjrt's multi-core path, but without
    donation so the same device buffers can be executed repeatedly for
    steady-state timing.
    """
    import jax
    from concourse import bass2jax
    from jax.experimental.shard_map import shard_map
    from jax.sharding import Mesh, NamedSharding, PartitionSpec

    bass2jax.install_neuronx_cc_hook()
    partition_name = nc.partition_id_tensor.name if nc.partition_id_tensor else None
    in_names, out_names, out_avals, zero_outs = [], [], [], []
    for alloc in nc.m.functions[0].allocations:
        if not isinstance(alloc, mybir.MemoryLocationSet):
            continue
        name = alloc.memorylocations[0].name
        if alloc.kind == "ExternalInput":
            if name != partition_name:
                in_names.append(name)
        elif alloc.kind == "ExternalOutput":
            out_names.append(name)
            shape = tuple(alloc.tensor_shape)
            dtype = mybir.dt.np(alloc.dtype)
            out_avals.append(jax.core.ShapedArray(shape, dtype))
            zero_outs.append(np.zeros(shape, dtype))
    n_params = len(in_names)
    all_names = in_names + out_names
    if partition_name is not None:
        all_names = all_names + [partition_name]

    def _body(*args):
        operands = list(args)
        if partition_name is not None:
            operands.append(bass2jax.partition_id_tensor())
        outs = bass2jax._bass_exec_p.bind(
            *operands,
            out_avals=tuple(out_avals),
            in_names=tuple(all_names),
            out_names=tuple(out_names),
            lowering_input_output_aliases=(),
            sim_require_finite=True,
            sim_require_nnan=True,
            nc=nc,
        )
        return tuple(outs)

    n_cores = len(in_maps)
    devices = jax.devices()[:n_cores]
    mesh = Mesh(np.asarray(devices), ("core",))
    in_specs = (PartitionSpec("core"),) * (n_params + len(out_names))
    out_specs = (PartitionSpec("core"),) * len(out_names)
    fn = jax.jit(
        shard_map(_body, mesh=mesh, in_specs=in_specs, out_specs=out_specs,
                  check_rep=False),
        keep_unused=True,
    )
    sh = NamedSharding(mesh, PartitionSpec("core"))
    concat = [
        np.concatenate([np.asarray(m[name]) for m in in_maps], axis=0)
        for name in in_names
    ] + [np.zeros((n_cores * z.shape[0], *z.shape[1:]), z.dtype) for z in zero_outs]
    dev_in = [jax.device_put(a, sh) for a in concat]
    return fn, dev_in, out_names, out_avals


def kernel(
    x_concepts_encoded, relevance_weights, bn_gamma, bn_beta, bn_mean, bn_var, W, b
):
    global _NC_CACHE, LAST_RESULT
    x_np = np.asarray(x_concepts_encoded, dtype=np.float32)
    w_np = np.asarray(relevance_weights, dtype=np.float32)
    if _NC_CACHE is None:
        _NC_CACHE = build_kernel()
    nc = _NC_CACHE
    in_maps = make_host_inputs(
        x_np,
        w_np,
        np.asarray(bn_gamma, dtype=np.float32),
        np.asarray(bn_beta, dtype=np.float32),
        np.asarray(bn_mean, dtype=np.float32),
        np.asarray(bn_var, dtype=np.float32),
        np.asarray(W, dtype=np.float32),
        np.asarray(b, dtype=np.float32),
    )
    trace = bool(int(os.environ.get("KERNEL_TRACE", "0")))
    LAST_RESULT = run_bass_kernel_spmd(
        nc, in_maps, core_ids=list(range(N_CORES)), trace=trace
    )
    out = np.concatenate(
        [LAST_RESULT.results[i]["out"].reshape(B_LOC, DO) for i in range(N_CORES)],
        axis=0,
    )
    return out



# revision 4
# speedup vs baseline: 8.1039x; 8.1039x over previous
"""Trainium2 Bass kernel for nn_Aggregator (BN1d + Swish + Linear + relevance-weighted head sum).

out[b, :] = sum_h w[b,h] * (silu(x[b,h,:] * inv + shift) @ W.T + bias)
          = (sum_h w[b,h] * silu(x[b,h,:] * inv + shift)) @ W.T + (sum_h w[b,h]) * bias

Data parallel over 8 NeuronCores: batch dim B split 8 ways, all params replicated.

v2 design — bf16 streaming + feature-transposed layout:
  - x is cast to bf16 host-side (tolerance 2e-2 >> bf16 error ~5e-3): halves
    the mandatory HBM read from 16.8 MB to 8.4 MB per core.
  - Features on PARTITIONS (d = c*128 + p), rows on the free dim. Per-core
    free layout per superblock (sb = 256 b-values): [c(4), nn(2), h(8), b(128)].
  - BN affine = one dual-scalar DVE/GPSIMD tensor_scalar per c-chunk
    (scale=inv_p, then add shift_p), 2x bf16 rate on DVE.
  - Swish = ONE big ACT instruction per superblock (N=8192/partition) —
    amortizes the 352-cycle ACT instruction overhead. ACT is the bottleneck
    engine at ~28.5 us/rep.
  - Weighted head-sum: DVE mult by w (broadcast across partitions) + 3-level
    contiguous tree add over h. No PE staircase, no transposes.
  - Linear: 16 small matmuls/sb (stationary W chunk [128d,128do], moving
    g [128d,128b]) accumulating over c in PSUM. PE ~5 us total.
  - bias: out = psum + b_do * sumw_b via one scalar_tensor_tensor per half.
  - Output stored bf16, transposed [do, b]; host unscrambles + upcasts.
"""

import os
from contextlib import ExitStack

import numpy as np

import concourse.bacc as bacc
import concourse.mybir as mybir
import concourse.tile as tile
from concourse.bass_utils import run_bass_kernel_spmd
from concourse.mybir import AluOpType

N_CORES = 8
B, H, D, DO = 8192, 8, 512, 256
B_LOC = B // N_CORES            # 1024 b-values per core
NSB = 4                         # superblocks of 256 b-values
SB_B = B_LOC // NSB             # 256 b per superblock
NN = 2                          # blocks of 128 b per superblock
NC_CHUNK = 4                    # feature chunks of 128
FREE = NC_CHUNK * NN * H * 128  # 8192 elems per partition per superblock
EPS = 1e-5
FP = mybir.dt.float32
BF = mybir.dt.bfloat16

# Which BN c-chunks run on GPSIMD (rest on DVE). Balancing knob.
GP_BN_CS = (2, 3)


def build_kernel(repeat: int = 1):
    """repeat>1 re-runs the whole superblock loop (same I/O) for slope timing."""
    nc = bacc.Bacc(
        "TRN2",
        target_bir_lowering=False,
        debug=False,
        num_devices=N_CORES,
    )

    x_d = nc.dram_tensor("x", (NSB, 128, FREE), BF, kind="ExternalInput")
    wb_d = nc.dram_tensor("wb", (128, NSB * NN * H * 128), BF, kind="ExternalInput")
    sumw_d = nc.dram_tensor("sumw", (128, NSB * NN * 128), FP, kind="ExternalInput")
    invT_d = nc.dram_tensor("invT", (128, NC_CHUNK), FP, kind="ExternalInput")
    shiftT_d = nc.dram_tensor("shiftT", (128, NC_CHUNK), FP, kind="ExternalInput")
    wt_d = nc.dram_tensor("wt", (128, 2 * NC_CHUNK * 128), BF, kind="ExternalInput")
    bvec_d = nc.dram_tensor("bvec", (128, 2), FP, kind="ExternalInput")
    out_d = nc.dram_tensor("out", (NSB, NN, 128, DO), BF, kind="ExternalOutput")

    with tile.TileContext(nc) as tc, ExitStack() as ctx:
        const = ctx.enter_context(tc.tile_pool(name="const", bufs=1))
        xpool = ctx.enter_context(tc.tile_pool(name="xin", bufs=3))

        # first superblock load precedes const loads in the SP FIFO except the
        # tiny BN params the first compute depends on
        invT = const.tile([128, NC_CHUNK], FP)
        nc.sync.dma_start(invT[:], invT_d.ap())
        shiftT = const.tile([128, NC_CHUNK], FP)
        nc.sync.dma_start(shiftT[:], shiftT_d.ap())
        xt0 = xpool.tile([128, FREE], BF, tag="xt")
        nc.sync.dma_start(xt0[:], x_d.ap()[0])
        wbc = const.tile([128, NSB * NN * H * 128], BF)
        nc.sync.dma_start(wbc[:], wb_d.ap())
        wt = const.tile([128, 2 * NC_CHUNK * 128], BF)
        nc.sync.dma_start(wt[:], wt_d.ap())
        sumw = const.tile([128, NSB * NN * 128], FP)
        nc.sync.dma_start(sumw[:], sumw_d.ap())
        bvec = const.tile([128, 2], FP)
        nc.sync.dma_start(bvec[:], bvec_d.ap())

        tpool = ctx.enter_context(tc.tile_pool(name="bn", bufs=2))
        hspool = ctx.enter_context(tc.tile_pool(name="hs", bufs=2))
        mpool = ctx.enter_context(tc.tile_pool(name="m", bufs=2))
        t4pool = ctx.enter_context(tc.tile_pool(name="t4", bufs=2))
        t2pool = ctx.enter_context(tc.tile_pool(name="t2", bufs=2))
        gpool = ctx.enter_context(tc.tile_pool(name="g", bufs=2))
        opool = ctx.enter_context(tc.tile_pool(name="o", bufs=4))
        pspool = ctx.enter_context(tc.tile_pool(name="ps", bufs=2, space="PSUM"))

        CH = FREE // NC_CHUNK  # 2048 elems per c-chunk per partition

        for rep in range(repeat):
          for sb in range(NSB):
            if sb == 0 and rep == 0:
                xt = xt0
            else:
                xt = xpool.tile([128, FREE], BF, tag="xt")
                nc.sync.dma_start(xt[:], x_d.ap()[sb])

            # BN affine per c-chunk: t = x * inv_p + shift_p
            t = tpool.tile([128, FREE], BF, tag="t")
            for c in range(NC_CHUNK):
                eng = nc.gpsimd if c in GP_BN_CS else nc.vector
                eng.tensor_scalar(
                    t[:, c * CH : (c + 1) * CH],
                    xt[:, c * CH : (c + 1) * CH],
                    invT[:, c : c + 1],
                    shiftT[:, c : c + 1],
                    AluOpType.mult,
                    AluOpType.add,
                )

            # Swish, one big ACT op
            hs = hspool.tile([128, FREE], BF, tag="hs")
            nc.scalar.activation(hs[:], t[:], mybir.ActivationFunctionType.Silu)

            # weighted by relevance: m = hs * w (w broadcast over partitions+c)
            m = mpool.tile([128, FREE], BF, tag="m")
            wslice = (
                wbc[:, sb * CH : (sb + 1) * CH]
                .unsqueeze(1)
                .broadcast_to([128, NC_CHUNK, CH])
            )
            nc.vector.tensor_tensor(
                m[:].rearrange("p (c r) -> p c r", c=NC_CHUNK),
                hs[:].rearrange("p (c r) -> p c r", c=NC_CHUNK),
                wslice,
                AluOpType.mult,
            )

            # head-sum: 3-level tree over h (cn = 2c+nn merged dim of 8)
            mv = m[:].rearrange("p (cn h b) -> p cn h b", cn=8, h=8)
            t4 = t4pool.tile([128, 8 * 4 * 128], BF, tag="t4")
            t4v = t4[:].rearrange("p (cn h b) -> p cn h b", cn=8, h=4)
            nc.vector.tensor_tensor(
                t4v, mv[:, :, 0:4, :], mv[:, :, 4:8, :], AluOpType.add
            )
            t2t = t2pool.tile([128, 8 * 2 * 128], BF, tag="t2")
            t2v = t2t[:].rearrange("p (cn h b) -> p cn h b", cn=8, h=2)
            nc.vector.tensor_tensor(
                t2v, t4v[:, :, 0:2, :], t4v[:, :, 2:4, :], AluOpType.add
            )
            g = gpool.tile([128, 8 * 128], BF, tag="g")
            gv = g[:].rearrange("p (cn h b) -> p cn h b", cn=8, h=1)
            nc.vector.tensor_tensor(
                gv, t2v[:, :, 0:1, :], t2v[:, :, 1:2, :], AluOpType.add
            )

            # Linear: accumulate over c into PSUM per (nn, half)
            ps = [
                pspool.tile([128, 128], FP, tag=f"ps{i}", name=f"ps{i}")
                for i in range(NN * 2)
            ]
            for c in range(NC_CHUNK):
                for half in range(2):
                    wsl = wt[:, (2 * c + half) * 128 : (2 * c + half + 1) * 128]
                    for n2 in range(NN):
                        cn = 2 * c + n2
                        nc.tensor.matmul(
                            ps[n2 * 2 + half][:],
                            wsl,
                            g[:, cn * 128 : (cn + 1) * 128],
                            start=(c == 0),
                            stop=(c == NC_CHUNK - 1),
                        )

            # bias: out = psum + b_do * sumw_b ; store bf16 via ACT HWDGE queue
            for n2 in range(NN):
                o = opool.tile([128, DO], BF, tag="o")
                for half in range(2):
                    nc.vector.scalar_tensor_tensor(
                        o[:, half * 128 : (half + 1) * 128],
                        sumw[:, (sb * NN + n2) * 128 : (sb * NN + n2 + 1) * 128],
                        bvec[:, half : half + 1],
                        ps[n2 * 2 + half][:],
                        AluOpType.mult,
                        AluOpType.add,
                    )
                nc.scalar.dma_start(out_d.ap()[sb][n2], o[:])

    nc.compile()
    return nc


def make_host_inputs(x_np, w_np, gamma, beta, mean, var, W, b):
    """Build the per-core input maps (host-side layout prep only)."""
    import ml_dtypes

    BFH = ml_dtypes.bfloat16
    inv = (gamma / np.sqrt(var + EPS)).astype(np.float32)
    shift = (beta - mean * inv).astype(np.float32)
    invT = np.ascontiguousarray(inv.reshape(NC_CHUNK, 128).T)
    shiftT = np.ascontiguousarray(shift.reshape(NC_CHUNK, 128).T)
    # Wt[p, (c*2+half)*128 + q] = W[half*128+q, c*128+p]
    wt = np.ascontiguousarray(
        W.astype(np.float32)
        .reshape(2, 128, NC_CHUNK, 128)  # [half, q, c, p]
        .transpose(3, 2, 0, 1)           # [p, c, half, q]
        .reshape(128, 2 * NC_CHUNK * 128)
        .astype(BFH)
    )
    bvec = np.ascontiguousarray(b.astype(np.float32).reshape(2, 128).T)

    xb = x_np.astype(BFH)  # round once, globally
    in_maps = []
    for core in range(N_CORES):
        b0 = core * B_LOC
        # x_t[sb, p, c, nn, h, bb] = x[b0 + sb*256 + nn*128 + bb, h, c*128 + p]
        x_core = xb[b0 : b0 + B_LOC].reshape(NSB, NN, 128, H, NC_CHUNK, 128)
        x_t = np.ascontiguousarray(
            x_core.transpose(0, 5, 4, 1, 3, 2).reshape(NSB, 128, FREE)
        )
        w_core = w_np[b0 : b0 + B_LOC].astype(np.float32)
        # wb[p, sb, nn, h, bb]
        w_r = (
            w_core.reshape(NSB, NN, 128, H).transpose(0, 1, 3, 2).reshape(-1)
        )
        wb = np.ascontiguousarray(
            np.broadcast_to(w_r.astype(BFH), (128, NSB * NN * H * 128))
        )
        sumw_r = w_core.sum(axis=1).astype(np.float32)  # [1024] = [sb, nn, bb]
        sumw = np.ascontiguousarray(np.broadcast_to(sumw_r, (128, NSB * NN * 128)))
        in_maps.append(
            {
                "x": x_t,
                "wb": wb,
                "sumw": sumw,
                "invT": invT,
                "shiftT": shiftT,
                "wt": wt,
                "bvec": bvec,
            }
        )
    return in_maps


_NC_CACHE = None
LAST_RESULT = None


def make_runner(nc, in_maps):
    """Build a reusable jitted SPMD callable with device-resident inputs.

    Mirrors bass2jax.run_bass_via_pjrt's multi-core path, but without
    donation so the same device buffers can be executed repeatedly for
    steady-state timing.
    """
    import jax
    from concourse import bass2jax
    from jax.experimental.shard_map import shard_map
    from jax.sharding import Mesh, NamedSharding, PartitionSpec

    bass2jax.install_neuronx_cc_hook()
    partition_name = nc.partition_id_tensor.name if nc.partition_id_tensor else None
    in_names, out_names, out_avals, zero_outs = [], [], [], []
    for alloc in nc.m.functions[0].allocations:
        if not isinstance(alloc, mybir.MemoryLocationSet):
            continue
        name = alloc.memorylocations[0].name
        if alloc.kind == "ExternalInput":
            if name != partition_name:
                in_names.append(name)
        elif alloc.kind == "ExternalOutput":
            out_names.append(name)
            shape = tuple(alloc.tensor_shape)
            dtype = mybir.dt.np(alloc.dtype)
            out_avals.append(jax.core.ShapedArray(shape, dtype))
            zero_outs.append(np.zeros(shape, dtype))
    n_params = len(in_names)
    all_names = in_names + out_names
    if partition_name is not None:
        all_names = all_names + [partition_name]

    def _body(*args):
        operands = list(args)
        if partition_name is not None:
            operands.append(bass2jax.partition_id_tensor())
        outs = bass2jax._bass_exec_p.bind(
            *operands,
            out_avals=tuple(out_avals),
            in_names=tuple(all_names),
            out_names=tuple(out_names),
            lowering_input_output_aliases=(),
            sim_require_finite=True,
            sim_require_nnan=True,
            nc=nc,
        )
        return tuple(outs)

    n_cores = len(in_maps)
    devices = jax.devices()[:n_cores]
    mesh = Mesh(np.asarray(devices), ("core",))
    in_specs = (PartitionSpec("core"),) * (n_params + len(out_names))
    out_specs = (PartitionSpec("core"),) * len(out_names)
    fn = jax.jit(
        shard_map(_body, mesh=mesh, in_specs=in_specs, out_specs=out_specs,
                  check_rep=False),
        keep_unused=True,
    )
    sh = NamedSharding(mesh, PartitionSpec("core"))
    concat = [
        np.concatenate([np.asarray(m[name]) for m in in_maps], axis=0)
        for name in in_names
    ] + [np.zeros((n_cores * z.shape[0], *z.shape[1:]), z.dtype) for z in zero_outs]
    dev_in = [jax.device_put(a, sh) for a in concat]
    return fn, dev_in, out_names, out_avals


def kernel(
    x_concepts_encoded, relevance_weights, bn_gamma, bn_beta, bn_mean, bn_var, W, b
):
    global _NC_CACHE, LAST_RESULT
    x_np = np.asarray(x_concepts_encoded, dtype=np.float32)
    w_np = np.asarray(relevance_weights, dtype=np.float32)
    if _NC_CACHE is None:
        _NC_CACHE = build_kernel()
    nc = _NC_CACHE
    in_maps = make_host_inputs(
        x_np,
        w_np,
        np.asarray(bn_gamma, dtype=np.float32),
        np.asarray(bn_beta, dtype=np.float32),
        np.asarray(bn_mean, dtype=np.float32),
        np.asarray(bn_var, dtype=np.float32),
        np.asarray(W, dtype=np.float32),
        np.asarray(b, dtype=np.float32),
    )
    trace = bool(int(os.environ.get("KERNEL_TRACE", "0")))
    LAST_RESULT = run_bass_kernel_spmd(
        nc, in_maps, core_ids=list(range(N_CORES)), trace=trace
    )
    outs = []
    for i in range(N_CORES):
        # out_d[sb, nn, p, half*128+bb] -> out[b_loc, do]
        o = np.asarray(LAST_RESULT.results[i]["out"]).astype(np.float32)
        o = o.reshape(NSB, NN, 128, 2, 128).transpose(0, 1, 4, 3, 2)
        outs.append(o.reshape(B_LOC, DO))
    return np.concatenate(outs, axis=0)
